# revision 3
# baseline (speedup 1.0000x reference)
"""GTE edge-attention kernel for trn2, 8 NeuronCores.

Strategy (edge-parallel, dst-sorted — per the sharding hint, edges and their
gathered src/dst features are partitioned per device):
  * Host sorts edges by dst and cuts them into 8 contiguous dst-ranges with
    balanced edge counts -> the segment softmax is fully local per core, no
    collectives.
  * Because scores are clipped to +-5, exp() cannot overflow, so the segment
    max subtraction cancels mathematically and is dropped; only segment SUMS
    remain.  Division by the softmax denominator is deferred to a per-node
    epilogue: wV = (sum_e ex*(V+score)) / (sum_e ex).
  * Host ships, per edge slot: x[src], x[dst], edge_attr rows, all
    pre-transposed to feature-major layout so they feed the tensor engine
    directly as lhsT.  The device does every projection (Q/K/V/E) per-edge
    with fp32r matmuls.
  * Edges are grouped into batches of 1024 (8 tiles of 128) whose dst span
    < 128 nodes and which end on node boundaries.  Per tile a one-hot
    matrix M^T[e,w] (w = dst - batch_window_start) turns the segment sums
    into one PE matmul accumulating over the batch; the [128,136] result is
    scatter-added into a DRAM accumulator with an indirect DMA (unique rows
    per instruction; rows beyond the batch's span are redirected to a dump
    region so concurrent scatters never collide).
"""
import os
import numpy as np
from contextlib import ExitStack

from concourse import bass, bacc, mybir, tile
from concourse.bass_utils import run_bass_kernel_spmd

P = 128
H = 8
D = 16
TILE = 128
BT = 8            # tiles per batch
BE = BT * TILE    # edges per batch
NCORES = 8

f32 = mybir.dt.float32
f32r = mybir.dt.float32r
i32 = mybir.dt.int32

LAST_EXEC_NS = None
LAST_RESULTS = None

_prog_cache = {}


def _build_program(NB, NTILES, EcP, NLP):
    nc = bacc.Bacc()
    xst = nc.declare_dram_parameter("xst", [P, EcP], f32, isOutput=False)
    xdt = nc.declare_dram_parameter("xdt", [P, EcP], f32, isOutput=False)
    eat = nc.declare_dram_parameter("eat", [P, EcP], f32, isOutput=False)
    woff = nc.declare_dram_parameter("woff", [P, NTILES], f32, isOutput=False)
    nodeid = nc.declare_dram_parameter("nodeid", [P, NB], i32, isOutput=False)
    qwz = nc.declare_dram_parameter("qwz", [P, 256], f32, isOutput=False)
    qbz = nc.declare_dram_parameter("qbz", [1, 256], f32, isOutput=False)
    kvw = nc.declare_dram_parameter("kvw", [P, 256], f32, isOutput=False)
    ewc = nc.declare_dram_parameter("ewc", [P, 256], f32, isOutput=False)
    ebc = nc.declare_dram_parameter("ebc", [1, 256], f32, isOutput=False)
    we = nc.declare_dram_parameter("we", [EcP, P], f32, isOutput=True)
    wvo = nc.declare_dram_parameter("wvo", [NLP, P], f32, isOutput=True)
    wv_acc = nc.dram_tensor("wv_acc", [NLP + P, 136], f32)

    with tile.TileContext(nc) as tc, ExitStack() as ctx:
        cpool = ctx.enter_context(tc.tile_pool(name="consts", bufs=1))
        sb = ctx.enter_context(tc.tile_pool(name="sb", bufs=2))
        ps = ctx.enter_context(tc.tile_pool(name="ps", bufs=2, space="PSUM"))

        # ---- constants ----
        qwz_t = cpool.tile([P, 256], dtype=f32r)
        nc.sync.dma_start(out=qwz_t[:], in_=qwz[:].bitcast(f32r))
        qbz_t = cpool.tile([1, 256], dtype=f32r)
        nc.sync.dma_start(out=qbz_t[:], in_=qbz[:].bitcast(f32r))
        kvw_t = cpool.tile([P, 256], dtype=f32r)
        nc.sync.dma_start(out=kvw_t[:], in_=kvw[:].bitcast(f32r))
        ewc_t = cpool.tile([P, 256], dtype=f32r)
        nc.sync.dma_start(out=ewc_t[:], in_=ewc[:].bitcast(f32r))
        ebc_t = cpool.tile([1, 256], dtype=f32r)
        nc.sync.dma_start(out=ebc_t[:], in_=ebc[:].bitcast(f32r))
        woff_t = cpool.tile([P, NTILES], dtype=f32)
        nc.sync.dma_start(out=woff_t[:], in_=woff[:])
        nodeid_t = cpool.tile([P, NB], dtype=i32)
        nc.sync.dma_start(out=nodeid_t[:], in_=nodeid[:])
        ones1 = cpool.tile([1, P], dtype=f32r)
        nc.vector.memset(ones1[:].bitcast(f32), 1.0)
        iota_i = cpool.tile([P, BT, TILE], dtype=i32)
        nc.gpsimd.iota(iota_i[:], pattern=[[0, BT], [1, TILE]], base=0,
                       channel_multiplier=0)
        iota_f = cpool.tile([P, BT, TILE], dtype=f32)
        nc.vector.tensor_copy(out=iota_f[:], in_=iota_i[:])
        zt = cpool.tile([P, 136], dtype=f32)
        nc.vector.memset(zt[:], 0.0)

        # ---- zero the accumulator ----
        for i in range((NLP + P) // P):
            nc.sync.dma_start(out=wv_acc[i * P:(i + 1) * P, :], in_=zt[:])

        # ---- main loop over batches ----
        for b in range(NB):
            e0 = b * BE
            xs_b = sb.tile([P, BE], dtype=f32r, tag="xs")
            nc.sync.dma_start(out=xs_b[:], in_=xst[:, e0:e0 + BE].bitcast(f32r))
            xd_b = sb.tile([P, BE], dtype=f32r, tag="xd")
            nc.sync.dma_start(out=xd_b[:], in_=xdt[:, e0:e0 + BE].bitcast(f32r))
            ea_b = sb.tile([P, BE], dtype=f32r, tag="ea")
            nc.sync.dma_start(out=ea_b[:], in_=eat[:, e0:e0 + BE].bitcast(f32r))

            # one-hot M^T for all 8 tiles at once
            mt_b = sb.tile([P, BT, TILE], dtype=f32r, tag="mt")
            nc.vector.tensor_tensor(
                out=mt_b[:], in0=iota_f[:],
                in1=woff_t[:, b * BT:(b + 1) * BT, None].to_broadcast([P, BT, TILE]),
                op=mybir.AluOpType.is_equal,
            )

            score_b = sb.tile([P, BT * TILE], dtype=f32, tag="score")
            t3_b = sb.tile([P, BT * TILE], dtype=f32, tag="t3")
            rhs_b = sb.tile([P, BT, 256], dtype=f32r, tag="rhs")
            s8_b = sb.tile([P, BT * H], dtype=f32, tag="s8")

            for j in range(BT):
                sl = slice(j * TILE, (j + 1) * TILE)
                kv_p = ps.tile([P, 256], dtype=f32, tag="kv", space="PSUM")
                nc.tensor.matmul(out=kv_p[:], lhsT=xs_b[:, sl], rhs=kvw_t[:],
                                 start=True, stop=True)
                q_p = ps.tile([P, 256], dtype=f32, tag="q", space="PSUM")
                nc.tensor.matmul(out=q_p[:], lhsT=xd_b[:, sl], rhs=qwz_t[:],
                                 start=True, stop=False)
                nc.tensor.matmul(out=q_p[:], lhsT=ones1[:], rhs=qbz_t[:],
                                 start=False, stop=True)
                ep_p = ps.tile([P, 256], dtype=f32, tag="ep", space="PSUM")
                nc.tensor.matmul(out=ep_p[:], lhsT=ea_b[:, sl], rhs=ewc_t[:],
                                 start=True, stop=False)
                nc.tensor.matmul(out=ep_p[:], lhsT=ones1[:], rhs=ebc_t[:],
                                 start=False, stop=True)

                qe_s = sb.tile([P, TILE], dtype=f32, tag="qe")
                nc.scalar.copy(out=qe_s[:], in_=q_p[:, 0:P])
                m1 = sb.tile([P, TILE], dtype=f32, tag="m1")
                nc.vector.tensor_tensor(out=m1[:], in0=kv_p[:, 0:P], in1=qe_s[:],
                                        op=mybir.AluOpType.mult)
                m2 = sb.tile([P, TILE], dtype=f32, tag="m2")
                nc.vector.tensor_tensor(out=m2[:], in0=m1[:], in1=ep_p[:, 0:P],
                                        op=mybir.AluOpType.mult)
                nc.vector.tensor_tensor(out=score_b[:, sl], in0=m2[:],
                                        in1=ep_p[:, P:256],
                                        op=mybir.AluOpType.add)
                nc.vector.tensor_tensor(out=t3_b[:, sl], in0=score_b[:, sl],
                                        in1=kv_p[:, P:256],
                                        op=mybir.AluOpType.add)
                nc.vector.tensor_reduce(
                    out=s8_b[:, j * H:(j + 1) * H],
                    in_=score_b[:, sl].rearrange("p (h d) -> p h d", h=H),
                    axis=mybir.AxisListType.X, op=mybir.AluOpType.add,
                )

            # clip & exp (scale 1/sqrt(D)=1/4 folded into activation scale)
            s8c_b = sb.tile([P, BT * H], dtype=f32, tag="s8c")
            nc.vector.tensor_scalar(
                out=s8c_b[:], in0=s8_b[:], scalar1=20.0, scalar2=-20.0,
                op0=mybir.AluOpType.min, op1=mybir.AluOpType.max,
            )
            rhs_v = rhs_b[:]  # [P, BT, 256]
            nc.scalar.activation(
                out=rhs_v[:, :, P:P + H], in_=s8c_b[:].rearrange("p (j h) -> p j h", j=BT),
                func=mybir.ActivationFunctionType.Exp, scale=0.25,
            )
            # msg = (V + score) * ex  (broadcast ex over D)
            nc.vector.tensor_tensor(
                out=rhs_v[:, :, 0:P].rearrange("p j (h d) -> p j h d", h=H),
                in0=t3_b[:].rearrange("p (j h d) -> p j h d", j=BT, h=H),
                in1=rhs_v[:, :, P:P + H][:, :, :, None].to_broadcast([P, BT, H, D]),
                op=mybir.AluOpType.mult,
            )

            wv_p = ps.tile([P, 256], dtype=f32, tag="wv", space="PSUM")
            for j in range(BT):
                nc.tensor.matmul(out=wv_p[:], lhsT=mt_b[:, j, :], rhs=rhs_b[:, j, :],
                                 start=(j == 0), stop=(j == BT - 1))
            wv_s = sb.tile([P, 136], dtype=f32, tag="wvs")
            nc.scalar.copy(out=wv_s[:], in_=wv_p[:, 0:136])
            nc.gpsimd.indirect_dma_start(
                out=wv_acc[:],
                out_offset=bass.IndirectOffsetOnAxis(ap=nodeid_t[:, b:b + 1], axis=0),
                in_=wv_s[:],
                in_offset=None,
                compute_op=mybir.AluOpType.add,
            )
            nc.sync.dma_start(
                out=we[e0:e0 + BE, :].rearrange("(j p) f -> p j f", j=BT),
                in_=score_b[:].rearrange("p (j f) -> p j f", j=BT),
            )

        # ---- epilogue: wv = num / (denom + 1e-16) ----
        for i in range(NLP // P):
            wv_t = sb.tile([P, 136], dtype=f32, tag="ewv")
            nc.sync.dma_start(out=wv_t[:], in_=wv_acc[i * P:(i + 1) * P, :])
            dp = sb.tile([P, H], dtype=f32, tag="edp")
            nc.vector.tensor_scalar_add(out=dp[:], in0=wv_t[:, P:P + H], scalar1=1e-16)
            rp = sb.tile([P, H], dtype=f32, tag="erp")
            nc.vector.reciprocal(out=rp[:], in_=dp[:])
            o_t = sb.tile([P, P], dtype=f32, tag="eot")
            nc.vector.tensor_tensor(
                out=o_t[:].rearrange("p (h d) -> p h d", h=H),
                in0=wv_t[:, 0:P].rearrange("p (h d) -> p h d", h=H),
                in1=rp[:, :, None].to_broadcast([P, H, D]),
                op=mybir.AluOpType.mult,
            )
            nc.sync.dma_start(out=wvo[i * P:(i + 1) * P, :], in_=o_t[:])
    nc.compile()
    return nc


def prepare(x, edge_attr, edge_index, QW, Qb, KW, EW, Eb, VW, num_nodes):
    x = np.asarray(x, dtype=np.float32)
    edge_attr = np.asarray(edge_attr, dtype=np.float32)
    ei = np.asarray(edge_index)
    src = ei[0].astype(np.int64)
    dst = ei[1].astype(np.int64)
    QW = np.asarray(QW, np.float32); Qb = np.asarray(Qb, np.float32)
    KW = np.asarray(KW, np.float32); EW = np.asarray(EW, np.float32)
    Eb = np.asarray(Eb, np.float32); VW = np.asarray(VW, np.float32)
    N = int(num_nodes)
    E = src.shape[0]

    # ---- sort edges by dst ----
    perm = np.argsort(dst, kind="stable")
    dst_s = dst[perm]
    src_s = src[perm]
    counts = np.bincount(dst_s, minlength=N)
    cum = np.concatenate([[0], np.cumsum(counts)])  # cum[n] = #edges with dst<n

    # ---- core cuts on node boundaries, balanced by edge count ----
    node_cut = [0]
    for c in range(1, NCORES):
        node_cut.append(int(np.searchsorted(cum, c * E / NCORES)))
    node_cut.append(N)

    # ---- per-core batching ----
    cores = []
    for c in range(NCORES):
        n0, n1 = node_cut[c], node_cut[c + 1]
        e0c, e1c = int(cum[n0]), int(cum[n1])
        batches = []  # (pos, upto, w0)
        pos = e0c
        while pos < e1c:
            w0 = int(dst_s[pos])
            limit_e = int(cum[min(w0 + P, n1)])
            cand = pos + BE
            if cand < min(limit_e, e1c):
                cand = int(cum[dst_s[cand]])
                upto = cand
            else:
                upto = min(limit_e, e1c)
            assert upto > pos, "batching stalled (node degree too large?)"
            assert upto - pos <= BE
            batches.append((pos, upto, w0))
            pos = upto
        cores.append((n0, n1, batches))

    NB = max(len(b) for _, _, b in cores)
    EcP = NB * BE
    NTILES = NB * BT
    NLP = ((max(n1 - n0 for n0, n1, _ in cores) + P - 1) // P) * P
    DUMP = NLP

    # ---- weights (column permutation only) ----
    qwz = np.concatenate([QW, np.zeros((P, P), np.float32)], axis=1)
    qbz = np.concatenate([Qb, np.zeros(P, np.float32)])[None, :]
    kvw = np.concatenate([KW, VW], axis=1)
    ew3 = EW.reshape(P, H, 2 * D)
    ewc = np.concatenate([ew3[:, :, :D].reshape(P, P), ew3[:, :, D:].reshape(P, P)], axis=1)
    eb3 = Eb.reshape(H, 2 * D)
    ebc = np.concatenate([eb3[:, :D].ravel(), eb3[:, D:].ravel()])[None, :]
    ewc = np.ascontiguousarray(ewc); ebc = np.ascontiguousarray(ebc)

    # ---- per-core input arrays ----
    in_maps = []
    slot_edges = []
    wincol = np.arange(P, dtype=np.int64)
    for c in range(NCORES):
        n0, n1, batches = cores[c]
        nb_real = len(batches)
        slot_edge = np.full(EcP, -1, np.int64)
        woff_flat = np.full(EcP, 255.0, np.float32)
        nodeid_c = np.empty((NB, P), np.int64)
        for i, (pos, upto, w0) in enumerate(batches):
            L = upto - pos
            slot_edge[i * BE:i * BE + L] = np.arange(pos, upto)
            woff_flat[i * BE:i * BE + L] = (dst_s[pos:upto] - w0).astype(np.float32)
            span_end = batches[i + 1][2] - n0 if i + 1 < nb_real else n1 - n0
            w0L = w0 - n0
            ids = w0L + wincol
            bad = (ids >= span_end) | (ids >= NLP)
            ids = np.where(bad, DUMP + wincol, ids)
            nodeid_c[i] = ids
        for i in range(nb_real, NB):
            nodeid_c[i] = DUMP + wincol

        sidx = np.where(slot_edge >= 0, slot_edge, 0)
        real = slot_edge >= 0
        s_src = np.where(real, src_s[sidx], 0)
        s_dst = np.where(real, dst_s[sidx], 0)
        xst_c = np.ascontiguousarray(x[s_src].T)
        xdt_c = np.ascontiguousarray(x[s_dst].T)
        eat_c = np.ascontiguousarray(edge_attr[np.where(real, perm[sidx], 0)].T)
        woff_c = np.ascontiguousarray(woff_flat.reshape(NTILES, P).T)
        nodeid_dev = np.ascontiguousarray(nodeid_c.T.astype(np.int32))
        in_maps.append(dict(
            xst=xst_c, xdt=xdt_c, eat=eat_c, woff=woff_c, nodeid=nodeid_dev,
            qwz=qwz, qbz=qbz, kvw=kvw, ewc=ewc, ebc=ebc,
        ))
        slot_edges.append((slot_edge, real))

    # ---- build ----
    key = (NB, NTILES, EcP, NLP)
    if key not in _prog_cache:
        _prog_cache[key] = _build_program(*key)
    nc = _prog_cache[key]
    meta = dict(N=N, E=E, cores=cores, slot_edges=slot_edges, perm=perm)
    return nc, in_maps, meta


def assemble(res, meta):
    N, E = meta["N"], meta["E"]
    cores, slot_edges, perm = meta["cores"], meta["slot_edges"], meta["perm"]
    wV = np.empty((N, P), np.float32)
    wE = np.empty((E, P), np.float32)
    for c in range(NCORES):
        n0, n1, _ = cores[c]
        wV[n0:n1] = res[c]["wvo"][: n1 - n0]
        slot_edge, real = slot_edges[c]
        orig = perm[slot_edge[real]]
        wE[orig] = res[c]["we"][real]
    return wV.reshape(N, H, D), wE


def kernel(**inputs):
    global LAST_EXEC_NS, LAST_RESULTS
    nc, in_maps, meta = prepare(**inputs)
    out = run_bass_kernel_spmd(nc, in_maps, list(range(NCORES)))
    LAST_EXEC_NS = out.exec_time_ns
    LAST_RESULTS = out
    return assemble(out.results, meta)
